# revision 15
# baseline (speedup 1.0000x reference)
"""Windowed (patch) attention kernel for 8 Trainium2 NeuronCores.

Problem: serialized point-cloud attention.
  qkv = feat @ Wqkv + bqkv ; qkv = qkv[order] -> windows of 256 rows
  per-window, per-head softmax attention ; out = attn_out[inverse] @ Wproj + bproj

Distribution strategy (per sharding hint): the permutation `order` is applied
host-side while sharding, so each core receives its 32 windows' rows already
gathered and channel-major (transposed).  All FLOPs (QKV proj, attention,
output proj) run on-device in bf16 with f32 PSUM accumulation.  `inverse`
scatter + bias adds are applied host-side (exact; row permutation commutes
with the row-wise projection, softmax is shift-invariant so the k-bias
cancels and the v-bias contributes bv @ Wproj to every row).

Schedule: the attention inner loop is software-pipelined one head-pair (hp)
stage deep: the PE stream per stage is [scores(s) | fillers+attnV(s-1)], so
the serial latency chain scores -> (sem) -> exp.hh0 -> exp.hh1 -> (sem) ->
attnV (~1.8us on the scalar engine) hides under a full stage (~2.1us) of PE
work instead of stalling the in-order PE queue.  Without the delay the
steady-state period is set by that latency chain (~2.46us/stage measured),
not by PE throughput (~2.13us/stage).
"""

import numpy as np
import ml_dtypes

import concourse.mybir as mybir
from concourse import bacc
from concourse.tile import TileContext
from concourse.bass_utils import run_bass_kernel_spmd

N = 65536
C = 512
H = 8
KW = 256          # window size
SCALE = 0.125
NCORES = 8
ROWS = N // NCORES        # 8192 rows per core
NWIN = ROWS // KW         # 32 windows per core
D = C // H                # 64 head dim

BF16 = mybir.dt.bfloat16
F32 = mybir.dt.float32


def build_nc():
    nc = bacc.Bacc("TRN2", target_bir_lowering=False, debug=False, num_devices=NCORES)

    # host pre-rearranged layouts: every DMA line is 2-4KB contiguous
    xt = nc.declare_dram_parameter("xt", [128, NWIN // 2, 4, 2 * KW], BF16, isOutput=False)
    wqkv = nc.declare_dram_parameter("wqkv", [128, 6, 4, 256], BF16, isOutput=False)
    wproj = nc.declare_dram_parameter("wproj", [C, C], BF16, isOutput=False)
    # bf16 output: halves the store traffic + tail drain; the host converts
    # back to f32 (rounding adds ~1e-3 rel err, budget is 2e-2)
    out = nc.declare_dram_parameter("out", [ROWS, C], BF16, isOutput=True)

    with TileContext(nc, pool_alloc_mode="queue") as tc:
        with (
            tc.tile_pool(name="const", bufs=1) as const,
            tc.tile_pool(name="sb", bufs=4) as sb,
            tc.tile_pool(name="outp", bufs=8) as outp,
            tc.tile_pool(name="eh", bufs=3) as eh,
            # one unified 2KB-slot PSUM ring (6 banks) for qkT accs, scores,
            # attnV outs, warmup and tail transposes; the scores tiles need
            # ~4 slots of depth on their own (a slot frees only at exp(s)
            # completion, ~1.8us after scores(s)) and with only 4 shared
            # slots the scores dispatch stalled ~100ns/stage on the ring.
            tc.tile_pool(name="pqs", bufs=6, space="PSUM") as pqs,
            tc.tile_pool(name="pvf", bufs=2, space="PSUM") as pvf,
        ):
            # --- persistent tiles: weights + identity -----------------------
            wq_sb = const.tile([128, 6, 4, 256], BF16)
            wp_sb = const.tile([128, 4, C], BF16)
            # split weight loads so the first matmul group isn't gated on the
            # whole 3 MB weight transfer; alternate between the two hardware
            # DGE queues (sync/scalar) -- serial on one ring the later chunks
            # arrived after the v-projection needed them (~1.8us PE stall)
            for mc in range(6):
                eng = nc.sync if mc % 2 == 0 else nc.scalar
                eng.dma_start(out=wq_sb[:, mc], in_=wqkv[:, mc])
            nc.scalar.dma_start(out=wp_sb, in_=wproj.rearrange("(j p) c -> p j c", p=128))

            # warm-up: keep the PE busy (and HAM un-throttled) while the
            # first weight/xt DMAs are in flight; results are discarded.
            junk = const.tile([128, C], BF16)
            nc.vector.memset(junk, 0.0)
            wup = pqs.tile([128, C], F32, tag="qs", name="wup")
            for _ in range(8):
                nc.tensor.matmul(wup, lhsT=junk[:, 0:128], rhs=junk, start=True, stop=True)
            GW = 2 * KW  # 2 windows per group: N=512 matmuls for qkv

            def xt_load(wg):
                """Prefetch a group's x^T slab (issued one group early)."""
                xt_g = sb.tile([128, 4, GW], BF16, name="xt_g")
                nc.gpsimd.dma_start(out=xt_g, in_=xt[:, wg])
                return xt_g

            def qkv_emit(wg, xt_g):
                """Emit qkv projection for group wg; yields after each PE matmul.
                First yield delivers (qkT, v_sb) tile handles."""
                # qk^T: q,k channels (1024) chunked by 128 -> [128, 8, 512]
                qkT = sb.tile([128, 8, GW], BF16, name="qkT")
                # v natural layout with ones column: [128rows, rc, head, 65]
                v_sb = sb.tile([128, 4, H, D + 1], BF16, name="v_sb")
                nc.vector.memset(v_sb[:, :, :, D:D + 1], 1.0)
                yield (qkT, v_sb)
                for m in range(8):
                    acc = pqs.tile([128, GW], F32, tag="qs", name="acc_qk")
                    for j in range(4):
                        nc.tensor.matmul(
                            acc,
                            lhsT=wq_sb[:, m // 2, j,
                                       (m % 2) * 128:(m % 2) * 128 + 128],
                            rhs=xt_g[:, j, :],
                            start=(j == 0),
                            stop=(j == 3),
                        )
                        yield None
                    nc.vector.tensor_copy(qkT[:, m, :], acc)
                for rc in range(4):
                    # emit the previous rc's deferred v-copy before taking
                    # another pvf slot (bufs=2 ring: the copy must be emitted
                    # before its slot can be recycled)
                    flush_sc()
                    acc = pvf.tile([128, C], F32, tag="vf", name="acc_v")
                    for j in range(4):
                        nc.tensor.matmul(
                            acc,
                            lhsT=xt_g[:, j, rc * 128:(rc + 1) * 128],
                            rhs=wq_sb[:, 4:6, j, :],
                            start=(j == 0),
                            stop=(j == 3),
                        )
                        yield None
                    # [128, 512] viewed as (H, D) -> strided into (H, 65)
                    # slots on the scalar engine (gpsimd can't read PSUM).
                    # Deferred: flushed right AFTER a stage's exp pair so the
                    # copy never sits between the two exps -- the exp pair's
                    # completion is the attnV critical path.
                    pending_sc.append((v_sb[:, rc, :, 0:D],
                                       acc.rearrange("p (h d) -> p h d", h=H)))

            pending_proj = []
            pending_sc = []

            def flush_sc():
                while pending_sc:
                    dst, src = pending_sc.pop(0)
                    nc.scalar.copy(dst, src)

            def proj_emit():
                """Emit the oldest pending projection (deferred so the attnT
                DMA-transpose has slack)."""
                attnT, r0 = pending_proj.pop(0)
                for rc in range(2):
                    acc = pvf.tile([128, C], F32, tag="vf", name="acc_p")
                    for cc in range(4):
                        nc.tensor.matmul(
                            acc,
                            lhsT=attnT[:, cc, rc * 128:(rc + 1) * 128],
                            rhs=wp_sb[:, cc, :],
                            start=(cc == 0),
                            stop=(cc == 3),
                        )
                        yield None
                    o_sb = outp.tile([128, C], BF16, name="o_sb")
                    if r0 >= (NWIN - 2) * KW:
                        # tail windows: half-column copy+DMA pairs so the
                        # final out DMA starts draining one copy earlier
                        for hc in range(2):
                            nc.vector.tensor_copy(
                                o_sb[:, hc * 256:(hc + 1) * 256],
                                acc[:, hc * 256:(hc + 1) * 256],
                            )
                            nc.sync.dma_start(
                                out=out[r0 + rc * 128:r0 + (rc + 1) * 128,
                                        hc * 256:(hc + 1) * 256],
                                in_=o_sb[:, hc * 256:(hc + 1) * 256],
                            )
                    else:
                        nc.vector.tensor_copy(o_sb, acc)
                        nc.gpsimd.dma_start(
                            out=out[r0 + rc * 128:r0 + (rc + 1) * 128, :], in_=o_sb
                        )

            # --- attention: one-stage-delayed software pipeline -------------
            # attn output tiles are per-window; a stage writes its hp slice.
            attn_box = {}      # wi_key -> attn tile

            def att_scores(st):
                """Emit scores matmuls + exp activations for stage st."""
                qkT = st["qkT"]
                qoff = st["qoff"]
                hp = st["hp"]
                scs = []
                exps = []
                for hh in range(2):
                    scs.append(pqs.tile([128, 2, KW], F32, tag="qs", name=f"sc{hh}"))
                    exps.append(eh.tile([128, 2, KW], BF16, tag="expT",
                                        name=f"expT{hh}", bufs=6))
                for kc in range(2):
                    for hh in range(2):
                        poff = hh * 64
                        nc.tensor.matmul(
                            scs[hh][:, kc, :],
                            lhsT=qkT[poff:poff + 64, 4 + hp,
                                     qoff + kc * 128:qoff + (kc + 1) * 128],
                            rhs=qkT[poff:poff + 64, hp, qoff:qoff + KW],
                            start=True,
                            stop=True,
                            tile_position=(poff, 0),
                        )
                        yield None
                for hh in range(2):
                    # expT[k,q] = exp(scale*scoresT); no max-sub
                    # (|scores*scale| bounded ~8 for these inputs)
                    nc.scalar.activation(
                        exps[hh], scs[hh],
                        mybir.ActivationFunctionType.Exp,
                        scale=SCALE,
                    )
                flush_sc()
                st["exps"] = exps

            def att_av(st, last=False):
                """Emit attnV + normalize for a (delayed) stage; yields 'v'
                before each short matmul so a long filler hides its
                LDWEIGHTS."""
                exps = st["exps"]
                v_sb = st["v_sb"]
                wi2 = st["wi2"]          # window index within group (0/1)
                hp = st["hp"]
                wkey = st["wkey"]
                if wkey not in attn_box:
                    attn_box[wkey] = sb.tile([128, 2, C], BF16, name="attn", bufs=5)
                attn = attn_box[wkey]
                # both heads' attn@V into one PSUM bank: (qc, hh, 65)
                ov = pqs.tile([128, 2, 2, D + 1], F32, tag="qs", name="ov")
                first = True
                for hh in range(2):
                    h = 2 * hp + hh
                    for qc in range(2):
                        for kc in range(2):
                            # no filler before the first attnV: its LDWEIGHTS
                            # hides under the preceding scores pair.  The
                            # spare filler goes AFTER the last attnV so the
                            # next stage's scores LDWEIGHTS hides under it.
                            if not first:
                                yield 'v'
                            first = False
                            nc.tensor.matmul(
                                ov[:, qc, hh, :],
                                lhsT=exps[hh][:, kc, qc * 128:(qc + 1) * 128],
                                rhs=v_sb[:, wi2 * 2 + kc, h, :],
                                start=(kc == 0),
                                stop=(kc == 1),
                            )
                yield 'v'
                # normalize by the ones-column result (both heads at once)
                rcp = eh.tile([128, 2, 2, 1], F32, tag="rcp", name="rcp")
                nc.vector.reciprocal(rcp, ov[:, :, :, D:D + 1])
                if last:
                    # tail: normalize on the scalar engine so the final
                    # windows don't queue behind vector work
                    for qc in range(2):
                        for hh in range(2):
                            nc.scalar.mul(
                                attn[:, qc,
                                     (2 * hp + hh) * D:(2 * hp + hh + 1) * D],
                                ov[:, qc, hh, 0:D],
                                rcp[:, qc, hh, 0:1],
                            )
                else:
                    nc.vector.tensor_mul(
                        attn[:, :, 2 * hp * D:(2 * hp + 2) * D].rearrange(
                            "p q (e d) -> p q e d", e=2
                        ),
                        ov[:, :, :, 0:D],
                        rcp.broadcast_to([128, 2, 2, D]),
                    )
                if hp == 3:
                    # window complete: transpose for the projection.  DMA
                    # transpose even at the tail: the PE-transpose path costs
                    # a ~2.1us pipeline DRAIN (transpose-mode switch) which is
                    # worse than the DMA-transpose latency it avoids.
                    r0 = st["r0"]
                    attnT = sb.tile([128, 4, KW], BF16, name="attnT", bufs=9)
                    for qc in range(2):
                        nc.sync.dma_start_transpose(
                            out=attnT[:, :, qc * 128:(qc + 1) * 128],
                            in_=attn[:, qc, :],
                        )
                    del attn_box[wkey]
                    pending_proj.append((attnT, r0))

            pend_att = [None]

            def att_emit(wg, qkT, v_sb, last=False):
                """Emit group wg's attention stages: per (wi, hp) slot, first
                scores(s), then attnV(s-1) (the delayed stage)."""
                for wi in range(2):
                    r0 = wg * GW + wi * KW
                    for hp in range(4):
                        st = {
                            "qkT": qkT, "v_sb": v_sb, "wi2": wi, "hp": hp,
                            "qoff": wi * KW, "r0": r0,
                            "wkey": (wg, wi),
                        }
                        yield from att_scores(st)
                        if pend_att[0] is not None:
                            yield from att_av(pend_att[0], last=last)
                        pend_att[0] = st

            def drain(g):
                for _ in g:
                    pass

            # software pipeline: qkv(wg) emission interleaves with the
            # attention of group wg-1.  Every 'v' slot pulls one long filler
            # matmul: first from the current qkv stream (48/group), then from
            # deferred projections (16/group) -- supply 64 exactly matches the
            # 64 attnV slots per group, so no attnV LDWEIGHTS is left exposed
            # in steady state.
            qg_box = [None]
            pg_box = [None]

            def pull_filler(depth):
                if qg_box[0] is not None:
                    if next(qg_box[0], StopIteration) is not StopIteration:
                        return
                    qg_box[0] = None
                while True:
                    if pg_box[0] is not None:
                        if next(pg_box[0], StopIteration) is not StopIteration:
                            return
                        pg_box[0] = None
                    if len(pending_proj) > depth:
                        pg_box[0] = proj_emit()
                    else:
                        return

            xt_cur = xt_load(0)
            xt_nxt = xt_load(1)
            qg = qkv_emit(0, xt_cur)
            tiles = next(qg)
            drain(qg)
            for wg in range(1, NWIN // 2):
                xt_cur = xt_nxt
                if wg + 1 < NWIN // 2:
                    xt_nxt = xt_load(wg + 1)
                qg_box[0] = qkv_emit(wg, xt_cur)
                new_tiles = next(qg_box[0])
                # pending depth ramps 2 -> 6 over the last groups so the tail
                # has ~6 windows of projections to cover the final attention
                depth = 2 if wg < 12 else wg - 9
                ag = att_emit(wg - 1, *tiles)
                while True:
                    tag = next(ag, StopIteration)
                    if tag is StopIteration:
                        break
                    if tag == 'v':
                        pull_filler(depth)
                if qg_box[0] is not None:
                    drain(qg_box[0])
                    qg_box[0] = None
                tiles = new_tiles
            # final group: the deferred projections fill all attnV slots
            ag = att_emit(NWIN // 2 - 1, *tiles, last=True)
            while True:
                tag = next(ag, StopIteration)
                if tag is StopIteration:
                    break
                if tag == 'v':
                    pull_filler(0)
            # flush the last delayed stage
            flush_sc()
            ag = att_av(pend_att[0], last=True)
            pend_att[0] = None
            while True:
                tag = next(ag, StopIteration)
                if tag is StopIteration:
                    break
                if tag == 'v':
                    pull_filler(0)
            if pg_box[0] is not None:
                drain(pg_box[0])
            while pending_proj:
                drain(proj_emit())
    nc.finalize()
    return nc


_NC_CACHE = None


def _get_nc():
    global _NC_CACHE
    if _NC_CACHE is None:
        _NC_CACHE = build_nc()
    return _NC_CACHE


def _prep_in_maps(feat, order, Wqkv, Wproj):
    xs = np.asarray(feat, dtype=np.float32)[np.asarray(order)]
    # wq[p, mc, j, c] = Wqkv[j*128+p, mc*256+c]: chunk-major so each DMA
    # line is 2KB contiguous
    wq = np.ascontiguousarray(
        np.asarray(Wqkv, dtype=np.float32)
        .reshape(4, 128, 6, 256).transpose(1, 2, 0, 3)
    ).astype(ml_dtypes.bfloat16)
    wp = np.asarray(Wproj, dtype=np.float32).astype(ml_dtypes.bfloat16)
    in_maps = []
    for m in range(NCORES):
        shard = xs[m * ROWS:(m + 1) * ROWS]
        xtb = np.ascontiguousarray(shard.T)  # [C, ROWS]
        # xt[p, g, j, r] = xtb[j*128+p, g*512+r]: per-group slabs are 4KB
        # contiguous per partition
        xtb = np.ascontiguousarray(
            xtb.reshape(4, 128, NWIN // 2, 512).transpose(1, 2, 0, 3)
        ).astype(ml_dtypes.bfloat16)
        in_maps.append({"xt": xtb, "wqkv": wq, "wproj": wp})
    return in_maps


def kernel(feat, order, inverse, Wqkv, bqkv, Wproj, bproj, _trace=False):
    nc = _get_nc()
    in_maps = _prep_in_maps(feat, order, Wqkv, Wproj)
    res = run_bass_kernel_spmd(nc, in_maps, core_ids=list(range(NCORES)), trace=_trace)
    serial = np.concatenate([r["out"] for r in res.results], axis=0)
    final = serial[np.asarray(inverse)].astype(np.float32)
    # biases (host-side, exact): v-bias rides through softmax (rows sum to 1)
    # as + bv @ Wproj ; k-bias cancels in softmax ; q-bias is zero by spec.
    total_bias = (
        np.asarray(bqkv, dtype=np.float32)[2 * C:3 * C] @ np.asarray(Wproj, dtype=np.float32)
        + np.asarray(bproj, dtype=np.float32)
    )
    out = final + total_bias[None, :]
    if _trace:
        return out.astype(np.float32), res
    return out.astype(np.float32)


# revision 16
# speedup vs baseline: 1.0493x; 1.0493x over previous
"""Windowed (patch) attention kernel for 8 Trainium2 NeuronCores.

Problem: serialized point-cloud attention.
  qkv = feat @ Wqkv + bqkv ; qkv = qkv[order] -> windows of 256 rows
  per-window, per-head softmax attention ; out = attn_out[inverse] @ Wproj + bproj

Distribution strategy (per sharding hint): the permutation `order` is applied
host-side while sharding, so each core receives its 32 windows' rows already
gathered and channel-major (transposed).  All FLOPs (QKV proj, attention,
output proj) run on-device in bf16 with f32 PSUM accumulation.  `inverse`
scatter + bias adds are applied host-side (exact; row permutation commutes
with the row-wise projection, softmax is shift-invariant so the k-bias
cancels and the v-bias contributes bv @ Wproj to every row).

Schedule: the attention inner loop is software-pipelined one head-pair (hp)
stage deep: the PE stream per stage is [scores(s) | fillers+attnV(s-1)], so
the serial latency chain scores -> (sem) -> exp.hh0 -> exp.hh1 -> (sem) ->
attnV (~1.8us on the scalar engine) hides under a full stage (~2.1us) of PE
work instead of stalling the in-order PE queue.  Without the delay the
steady-state period is set by that latency chain (~2.46us/stage measured),
not by PE throughput (~2.13us/stage).
"""

import numpy as np
import ml_dtypes

import concourse.mybir as mybir
from concourse import bacc
from concourse.tile import TileContext
from concourse.masks import make_identity
from concourse.bass_utils import run_bass_kernel_spmd

N = 65536
C = 512
H = 8
KW = 256          # window size
SCALE = 0.125
NCORES = 8
ROWS = N // NCORES        # 8192 rows per core
NWIN = ROWS // KW         # 32 windows per core
D = C // H                # 64 head dim

BF16 = mybir.dt.bfloat16
F32 = mybir.dt.float32


def build_nc():
    nc = bacc.Bacc("TRN2", target_bir_lowering=False, debug=False, num_devices=NCORES)

    # host pre-rearranged layouts: every DMA line is 2-4KB contiguous
    xt = nc.declare_dram_parameter("xt", [128, NWIN // 2, 4, 2 * KW], BF16, isOutput=False)
    wqkv = nc.declare_dram_parameter("wqkv", [128, 6, 4, 256], BF16, isOutput=False)
    wproj = nc.declare_dram_parameter("wproj", [C, C], BF16, isOutput=False)
    # bf16 output: halves the store traffic + tail drain; the host converts
    # back to f32 (rounding adds ~1e-3 rel err, budget is 2e-2)
    out = nc.declare_dram_parameter("out", [ROWS, C], BF16, isOutput=True)

    with TileContext(nc, pool_alloc_mode="queue") as tc:
        with (
            tc.tile_pool(name="const", bufs=1) as const,
            tc.tile_pool(name="sb", bufs=4) as sb,
            tc.tile_pool(name="outp", bufs=8) as outp,
            tc.tile_pool(name="eh", bufs=3) as eh,
            # one unified 2KB-slot PSUM ring (6 banks) for qkT accs, scores,
            # attnV outs, warmup and tail transposes; the scores tiles need
            # ~4 slots of depth on their own (a slot frees only at exp(s)
            # completion, ~1.8us after scores(s)) and with only 4 shared
            # slots the scores dispatch stalled ~100ns/stage on the ring.
            tc.tile_pool(name="pqs", bufs=6, space="PSUM") as pqs,
            tc.tile_pool(name="pvf", bufs=2, space="PSUM") as pvf,
        ):
            # --- persistent tiles: weights + identity -----------------------
            wq_sb = const.tile([128, 6, 4, 256], BF16)
            wp_sb = const.tile([128, 4, C], BF16)
            # split weight loads so the first matmul group isn't gated on the
            # whole 3 MB weight transfer
            for mc in range(6):
                nc.sync.dma_start(out=wq_sb[:, mc], in_=wqkv[:, mc])
            nc.sync.dma_start(out=wp_sb, in_=wproj.rearrange("(j p) c -> p j c", p=128))

            # warm-up: keep the PE busy (and HAM un-throttled) while the
            # first weight/xt DMAs are in flight; results are discarded.
            junk = const.tile([128, C], BF16)
            nc.vector.memset(junk, 0.0)
            wup = pqs.tile([128, C], F32, tag="qs", name="wup")
            for _ in range(13):
                nc.tensor.matmul(wup, lhsT=junk[:, 0:128], rhs=junk, start=True, stop=True)
            ident = const.tile([128, 128], BF16)
            GW = 2 * KW  # 2 windows per group: N=512 matmuls for qkv

            def xt_load(wg):
                """Prefetch a group's x^T slab (issued one group early)."""
                xt_g = sb.tile([128, 4, GW], BF16, name="xt_g")
                nc.gpsimd.dma_start(out=xt_g, in_=xt[:, wg])
                return xt_g

            def qkv_emit(wg, xt_g):
                """Emit qkv projection for group wg; yields after each PE matmul.
                First yield delivers (qkT, v_sb) tile handles."""
                # qk^T: q,k channels (1024) chunked by 128 -> [128, 8, 512]
                qkT = sb.tile([128, 8, GW], BF16, name="qkT")
                # v natural layout with ones column: [128rows, rc, head, 65]
                v_sb = sb.tile([128, 4, H, D + 1], BF16, name="v_sb")
                nc.vector.memset(v_sb[:, :, :, D:D + 1], 1.0)
                yield (qkT, v_sb)
                for m in range(8):
                    acc = pqs.tile([128, GW], F32, tag="qs", name="acc_qk")
                    for j in range(4):
                        nc.tensor.matmul(
                            acc,
                            lhsT=wq_sb[:, m // 2, j,
                                       (m % 2) * 128:(m % 2) * 128 + 128],
                            rhs=xt_g[:, j, :],
                            start=(j == 0),
                            stop=(j == 3),
                        )
                        yield None
                    nc.vector.tensor_copy(qkT[:, m, :], acc)
                for rc in range(4):
                    # emit the previous rc's deferred v-copy before taking
                    # another pvf slot (bufs=2 ring: the copy must be emitted
                    # before its slot can be recycled)
                    flush_sc()
                    acc = pvf.tile([128, C], F32, tag="vf", name="acc_v")
                    for j in range(4):
                        nc.tensor.matmul(
                            acc,
                            lhsT=xt_g[:, j, rc * 128:(rc + 1) * 128],
                            rhs=wq_sb[:, 4:6, j, :],
                            start=(j == 0),
                            stop=(j == 3),
                        )
                        yield None
                    # [128, 512] viewed as (H, D) -> strided into (H, 65)
                    # slots on the scalar engine (gpsimd can't read PSUM).
                    # Deferred: flushed right AFTER a stage's exp pair so the
                    # copy never sits between the two exps -- the exp pair's
                    # completion is the attnV critical path.
                    pending_sc.append((v_sb[:, rc, :, 0:D],
                                       acc.rearrange("p (h d) -> p h d", h=H)))

            pending_proj = []
            pending_sc = []

            def flush_sc():
                while pending_sc:
                    dst, src = pending_sc.pop(0)
                    nc.scalar.copy(dst, src)

            def proj_emit():
                """Emit the oldest pending projection (deferred so the attnT
                DMA-transpose has slack)."""
                attnT, r0 = pending_proj.pop(0)
                for rc in range(2):
                    acc = pvf.tile([128, C], F32, tag="vf", name="acc_p")
                    for cc in range(4):
                        nc.tensor.matmul(
                            acc,
                            lhsT=attnT[:, cc, rc * 128:(rc + 1) * 128],
                            rhs=wp_sb[:, cc, :],
                            start=(cc == 0),
                            stop=(cc == 3),
                        )
                        yield None
                    o_sb = outp.tile([128, C], BF16, name="o_sb")
                    if r0 >= (NWIN - 2) * KW:
                        # tail windows: half-column copy+DMA pairs so the
                        # final out DMA starts draining one copy earlier
                        for hc in range(2):
                            nc.vector.tensor_copy(
                                o_sb[:, hc * 256:(hc + 1) * 256],
                                acc[:, hc * 256:(hc + 1) * 256],
                            )
                            nc.sync.dma_start(
                                out=out[r0 + rc * 128:r0 + (rc + 1) * 128,
                                        hc * 256:(hc + 1) * 256],
                                in_=o_sb[:, hc * 256:(hc + 1) * 256],
                            )
                    else:
                        nc.vector.tensor_copy(o_sb, acc)
                        nc.gpsimd.dma_start(
                            out=out[r0 + rc * 128:r0 + (rc + 1) * 128, :], in_=o_sb
                        )

            # --- attention: one-stage-delayed software pipeline -------------
            # attn output tiles are per-window; a stage writes its hp slice.
            attn_box = {}      # wi_key -> attn tile

            def att_scores(st):
                """Emit scores matmuls + exp activations for stage st."""
                qkT = st["qkT"]
                qoff = st["qoff"]
                hp = st["hp"]
                scs = []
                exps = []
                for hh in range(2):
                    scs.append(pqs.tile([128, 2, KW], F32, tag="qs", name=f"sc{hh}"))
                    exps.append(eh.tile([128, 2, KW], BF16, tag="expT",
                                        name=f"expT{hh}", bufs=6))
                for kc in range(2):
                    for hh in range(2):
                        poff = hh * 64
                        nc.tensor.matmul(
                            scs[hh][:, kc, :],
                            lhsT=qkT[poff:poff + 64, 4 + hp,
                                     qoff + kc * 128:qoff + (kc + 1) * 128],
                            rhs=qkT[poff:poff + 64, hp, qoff:qoff + KW],
                            start=True,
                            stop=True,
                            tile_position=(poff, 0),
                        )
                        yield None
                for hh in range(2):
                    # expT[k,q] = exp(scale*scoresT); no max-sub
                    # (|scores*scale| bounded ~8 for these inputs)
                    nc.scalar.activation(
                        exps[hh], scs[hh],
                        mybir.ActivationFunctionType.Exp,
                        scale=SCALE,
                    )
                flush_sc()
                st["exps"] = exps

            def att_av(st, last=False):
                """Emit attnV + normalize for a (delayed) stage; yields 'v'
                before each short matmul so a long filler hides its
                LDWEIGHTS."""
                exps = st["exps"]
                v_sb = st["v_sb"]
                wi2 = st["wi2"]          # window index within group (0/1)
                hp = st["hp"]
                wkey = st["wkey"]
                if wkey not in attn_box:
                    attn_box[wkey] = sb.tile([128, 2, C], BF16, name="attn", bufs=5)
                attn = attn_box[wkey]
                # both heads' attn@V into one PSUM bank: (qc, hh, 65)
                ov = pqs.tile([128, 2, 2, D + 1], F32, tag="qs", name="ov")
                first = True
                for hh in range(2):
                    h = 2 * hp + hh
                    for qc in range(2):
                        for kc in range(2):
                            # no filler before the first attnV: its LDWEIGHTS
                            # hides under the preceding scores pair.  The
                            # spare filler goes AFTER the last attnV so the
                            # next stage's scores LDWEIGHTS hides under it.
                            if not first:
                                yield 'v'
                            first = False
                            nc.tensor.matmul(
                                ov[:, qc, hh, :],
                                lhsT=exps[hh][:, kc, qc * 128:(qc + 1) * 128],
                                rhs=v_sb[:, wi2 * 2 + kc, h, :],
                                start=(kc == 0),
                                stop=(kc == 1),
                            )
                yield 'v'
                # normalize by the ones-column result (both heads at once)
                rcp = eh.tile([128, 2, 2, 1], F32, tag="rcp", name="rcp")
                nc.vector.reciprocal(rcp, ov[:, :, :, D:D + 1])
                if last:
                    # tail: normalize on the scalar engine so the final
                    # windows don't queue behind vector work
                    for qc in range(2):
                        for hh in range(2):
                            nc.scalar.mul(
                                attn[:, qc,
                                     (2 * hp + hh) * D:(2 * hp + hh + 1) * D],
                                ov[:, qc, hh, 0:D],
                                rcp[:, qc, hh, 0:1],
                            )
                else:
                    nc.vector.tensor_mul(
                        attn[:, :, 2 * hp * D:(2 * hp + 2) * D].rearrange(
                            "p q (e d) -> p q e d", e=2
                        ),
                        ov[:, :, :, 0:D],
                        rcp.broadcast_to([128, 2, 2, D]),
                    )
                if hp == 3:
                    # window complete: transpose for the projection
                    r0 = st["r0"]
                    attnT = sb.tile([128, 4, KW], BF16, name="attnT", bufs=9)
                    if last:
                        # tail: PE transposes avoid the DMA-transpose latency
                        # when there is no filler left to hide it
                        for qc in range(2):
                            tp = pqs.tile([128, 4, 128], BF16, tag="qs",
                                          name=f"tp{qc}")
                            for cc in range(4):
                                nc.tensor.transpose(
                                    tp[:, cc, :],
                                    attn[:, qc, cc * 128:(cc + 1) * 128],
                                    ident,
                                )
                                yield None
                            nc.vector.tensor_copy(
                                attnT[:, :, qc * 128:(qc + 1) * 128], tp
                            )
                    else:
                        for qc in range(2):
                            nc.sync.dma_start_transpose(
                                out=attnT[:, :, qc * 128:(qc + 1) * 128],
                                in_=attn[:, qc, :],
                            )
                    del attn_box[wkey]
                    pending_proj.append((attnT, r0))

            pend_att = [None]

            def att_emit(wg, qkT, v_sb, last=False):
                """Emit group wg's attention stages: per (wi, hp) slot, first
                scores(s), then attnV(s-1) (the delayed stage)."""
                for wi in range(2):
                    r0 = wg * GW + wi * KW
                    for hp in range(4):
                        st = {
                            "qkT": qkT, "v_sb": v_sb, "wi2": wi, "hp": hp,
                            "qoff": wi * KW, "r0": r0,
                            "wkey": (wg, wi),
                        }
                        yield from att_scores(st)
                        if pend_att[0] is not None:
                            yield from att_av(pend_att[0], last=last)
                        pend_att[0] = st

            def drain(g):
                for _ in g:
                    pass

            # software pipeline: qkv(wg) emission interleaves with the
            # attention of group wg-1.  Every 'v' slot pulls one long filler
            # matmul: first from the current qkv stream (48/group), then from
            # deferred projections (16/group) -- supply 64 exactly matches the
            # 64 attnV slots per group, so no attnV LDWEIGHTS is left exposed
            # in steady state.
            qg_box = [None]
            pg_box = [None]

            def pull_filler(depth):
                if qg_box[0] is not None:
                    if next(qg_box[0], StopIteration) is not StopIteration:
                        return
                    qg_box[0] = None
                while True:
                    if pg_box[0] is not None:
                        if next(pg_box[0], StopIteration) is not StopIteration:
                            return
                        pg_box[0] = None
                    if len(pending_proj) > depth:
                        pg_box[0] = proj_emit()
                    else:
                        return

            xt_cur = xt_load(0)
            xt_nxt = xt_load(1)
            # identity for the tail PE-transposes; emitted after the xt
            # prefetches so its gpsimd ops don't delay the first transfers
            make_identity(nc, ident)
            qg = qkv_emit(0, xt_cur)
            tiles = next(qg)
            drain(qg)
            for wg in range(1, NWIN // 2):
                xt_cur = xt_nxt
                if wg + 1 < NWIN // 2:
                    xt_nxt = xt_load(wg + 1)
                qg_box[0] = qkv_emit(wg, xt_cur)
                new_tiles = next(qg_box[0])
                # pending depth ramps 2 -> 6 over the last groups so the tail
                # has ~6 windows of projections to cover the final attention
                depth = 2 if wg < 12 else wg - 9
                ag = att_emit(wg - 1, *tiles)
                while True:
                    tag = next(ag, StopIteration)
                    if tag is StopIteration:
                        break
                    if tag == 'v':
                        pull_filler(depth)
                if qg_box[0] is not None:
                    drain(qg_box[0])
                    qg_box[0] = None
                tiles = new_tiles
            # final group: the deferred projections fill all attnV slots
            ag = att_emit(NWIN // 2 - 1, *tiles, last=True)
            while True:
                tag = next(ag, StopIteration)
                if tag is StopIteration:
                    break
                if tag == 'v':
                    pull_filler(0)
            # flush the last delayed stage
            flush_sc()
            ag = att_av(pend_att[0], last=True)
            pend_att[0] = None
            while True:
                tag = next(ag, StopIteration)
                if tag is StopIteration:
                    break
                if tag == 'v':
                    pull_filler(0)
            if pg_box[0] is not None:
                drain(pg_box[0])
            while pending_proj:
                drain(proj_emit())
    nc.finalize()
    return nc


_NC_CACHE = None


def _get_nc():
    global _NC_CACHE
    if _NC_CACHE is None:
        _NC_CACHE = build_nc()
    return _NC_CACHE


def _prep_in_maps(feat, order, Wqkv, Wproj):
    xs = np.asarray(feat, dtype=np.float32)[np.asarray(order)]
    # wq[p, mc, j, c] = Wqkv[j*128+p, mc*256+c]: chunk-major so each DMA
    # line is 2KB contiguous
    wq = np.ascontiguousarray(
        np.asarray(Wqkv, dtype=np.float32)
        .reshape(4, 128, 6, 256).transpose(1, 2, 0, 3)
    ).astype(ml_dtypes.bfloat16)
    wp = np.asarray(Wproj, dtype=np.float32).astype(ml_dtypes.bfloat16)
    in_maps = []
    for m in range(NCORES):
        shard = xs[m * ROWS:(m + 1) * ROWS]
        xtb = np.ascontiguousarray(shard.T)  # [C, ROWS]
        # xt[p, g, j, r] = xtb[j*128+p, g*512+r]: per-group slabs are 4KB
        # contiguous per partition
        xtb = np.ascontiguousarray(
            xtb.reshape(4, 128, NWIN // 2, 512).transpose(1, 2, 0, 3)
        ).astype(ml_dtypes.bfloat16)
        in_maps.append({"xt": xtb, "wqkv": wq, "wproj": wp})
    return in_maps


def kernel(feat, order, inverse, Wqkv, bqkv, Wproj, bproj, _trace=False):
    nc = _get_nc()
    in_maps = _prep_in_maps(feat, order, Wqkv, Wproj)
    res = run_bass_kernel_spmd(nc, in_maps, core_ids=list(range(NCORES)), trace=_trace)
    serial = np.concatenate([r["out"] for r in res.results], axis=0)
    final = serial[np.asarray(inverse)].astype(np.float32)
    # biases (host-side, exact): v-bias rides through softmax (rows sum to 1)
    # as + bv @ Wproj ; k-bias cancels in softmax ; q-bias is zero by spec.
    total_bias = (
        np.asarray(bqkv, dtype=np.float32)[2 * C:3 * C] @ np.asarray(Wproj, dtype=np.float32)
        + np.asarray(bproj, dtype=np.float32)
    )
    out = final + total_bias[None, :]
    if _trace:
        return out.astype(np.float32), res
    return out.astype(np.float32)
